# revision 8
# baseline (speedup 1.0000x reference)
"""Bass/Trainium2 kernel for nn_Attention (B=2, N=2048, C=768, H=12).

Sharding: 8 cores = 2 batches x 4 head-triples. Core (b, hh) computes
Q/K/V projections for heads {3hh, 3hh+1, 3hh+2} over the full 2048-token
sequence of batch b, attention for those heads, and the partial output
projection y_partial = (attn_out * gate) @ w_proj[rows of those heads].
Host sums the 4 fp32 partials per batch and adds b_proj.

v3 schedule (driven by ntff traces of v1/v2):
- Score tiles are [128, 1024] (2 PSUM banks): A-scores of key tile kt in
  the low half, B-scores of the same kt in the high half, so the two
  score matmuls of a tile are the natural (0,0)/(64,0) row-group pair
  (concurrent fills on separate XBUSes) and each exp instruction covers
  1024 els/partition (halves the fixed cost of ACT/DVE).
- V tiles are padded to [128, 128] per head (ones col at 64, junk
  above): 128-column stationary tiles get Fast Weight Load.
- y projection accumulates pair+C pieces in one PSUM chain and DMAs the
  [128, 384] fp32 result straight to DRAM (out is fp32; host sums) --
  no ysb staging copies, no tail add/copy ladder.
- normalize: denominator rows (PSUM row 64) are DMA'd to SBUF (DMA
  queue, not ACT/DVE), reciprocals batched (A+B in one DVE instr),
  broadcast on GpSimd, one [64,512] mul per head evacuates/normalizes.
- exp split: per block 24 [128,1024] tiles; ACT takes 14, DVE 10
  (DVE also carries normalize) -- both land at the PE floor.
- Warmup matmuls on a memset tile (no DMA dependency) plus a few on
  wk_s once it lands keep HAM at 8/8 through the projections; the ACT
  Exp table preloads during the DMA window.
"""

import numpy as np
import ml_dtypes

B, N, C = 2, 2048, 768
H = 12
DH = C // H
SCALE = DH**-0.5
P = 128
HL = 3  # heads per core
KJ = C // P  # 6 contraction tiles over C
KT = N // P  # 16 key tiles
NB = N // 512  # 4 query blocks / x chunks
VW = HL * (DH + 1)  # 195 v columns (ones col per head)

EXP_C1 = 128.0 / float(np.log(2.0))
EXP_C2 = 16256.0 - 5.5

NCORES = 8
TRACE = False  # test.py flips this to profile
LAST_RESULT = None

_BF16 = ml_dtypes.bfloat16

_nc_cache = None


def _build_nc():
    from contextlib import ExitStack

    import concourse.tile as tile
    from concourse import bacc, mybir

    dt = mybir.dt
    F32, BF16, I16 = dt.float32, dt.bfloat16, dt.int16
    AF = mybir.ActivationFunctionType
    ALU = mybir.AluOpType

    nc = bacc.Bacc("TRN2", target_bir_lowering=False, num_devices=NCORES)

    xt = [
        nc.dram_tensor(f"xt{n}", [P, KJ * 512], BF16, kind="ExternalInput")
        for n in range(NB)
    ]
    wq = nc.dram_tensor("wq", [P, KJ * P], BF16, kind="ExternalInput")
    wk = nc.dram_tensor("wk", [P, KJ * P], BF16, kind="ExternalInput")
    wc = nc.dram_tensor("wc", [P, KJ * P], BF16, kind="ExternalInput")
    wv = nc.dram_tensor("wv", [P, KJ * VW], BF16, kind="ExternalInput")
    wpp = nc.dram_tensor("wpp", [P, C], BF16, kind="ExternalInput")  # pair rows
    wpc = nc.dram_tensor("wpc", [P, C], BF16, kind="ExternalInput")  # head C rows x2
    out = nc.dram_tensor("out", [N, C], BF16, kind="ExternalOutput")

    with tile.TileContext(nc) as tc, ExitStack() as ctx:
        ps_pool = ctx.enter_context(tc.tile_pool(name="persist", bufs=1))

        xT = [
            ps_pool.tile([P, KJ, 512], BF16, tag=f"xT{n}", name=f"xT{n}")
            for n in range(NB)
        ]
        wq_s = ps_pool.tile([P, KJ * P], BF16, tag="wq")
        wk_s = ps_pool.tile([P, KJ * P], BF16, tag="wk")
        wc_s = ps_pool.tile([P, KJ * P], BF16, tag="wc")
        wv_s = ps_pool.tile([P, KJ * VW], BF16, tag="wv")
        wpp_s = ps_pool.tile([P, C], BF16, tag="wpp")
        wpc_s = ps_pool.tile([P, C], BF16, tag="wpc")
        qTp = [ps_pool.tile([P, 512], BF16, tag=f"qTp{n}", name=f"qTp{n}") for n in range(NB)]
        kTp = [ps_pool.tile([P, 512], BF16, tag=f"kTp{n}", name=f"kTp{n}") for n in range(NB)]
        qTc = [ps_pool.tile([P, 512], BF16, tag=f"qTc{n}", name=f"qTc{n}") for n in range(NB)]
        kTc = [ps_pool.tile([P, 512], BF16, tag=f"kTc{n}", name=f"kTc{n}") for n in range(NB)]
        # V: per key tile, [128, 3*128]: head h at cols h*128..h*128+64
        # (64 dh + ones col); cols 65-127 of each head are never-read junk
        vsb = [ps_pool.tile([P, HL * P], BF16, tag=f"v{t}", name=f"v{t}") for t in range(KT)]
        otP = [
            ps_pool.tile([P, 512], BF16, tag=f"otP{q}", name=f"otP{q}") for q in range(NB)
        ]
        otC = [
            ps_pool.tile([P, 512], BF16, tag=f"otC{q}", name=f"otC{q}") for q in range(NB)
        ]

        def kslice(kTx, kt):
            return kTx[kt // 4][:, (kt % 4) * P : (kt % 4 + 1) * P]

        def vhead(t, h):
            return vsb[t][:, h * P : (h + 1) * P]

        # ---- input loads (one HWDGE ring, FIFO) ----
        nc.sync.dma_start(wk_s[:], wk[:])
        nc.sync.dma_start(xT[0][:], xt[0][:].rearrange("p (j n) -> p j n", n=512))
        nc.sync.dma_start(wc_s[:], wc[:])
        nc.sync.dma_start(wv_s[:], wv[:])
        nc.sync.dma_start(wq_s[:], wq[:])
        nc.sync.dma_start(xT[1][:], xt[1][:].rearrange("p (j n) -> p j n", n=512))
        nc.sync.dma_start(xT[2][:], xt[2][:].rearrange("p (j n) -> p j n", n=512))
        nc.sync.dma_start(xT[3][:], xt[3][:].rearrange("p (j n) -> p j n", n=512))
        nc.sync.dma_start(wpp_s[:], wpp[:])
        nc.sync.dma_start(wpc_s[:], wpc[:])

        with (
            tc.tile_pool(name="st", bufs=2, space="PSUM") as stp,
            tc.tile_pool(name="ot", bufs=2, space="PSUM") as otp,
            tc.tile_pool(name="yp", bufs=2, space="PSUM") as ypp,
            tc.tile_pool(name="pexp", bufs=12) as pexp,
        ):
            # ---- ACT Exp table preload (hidden under the DMA window) ----
            tw = pexp.tile([1, 8], F32, tag="rc", bufs=6, name="twarm")
            nc.vector.memset(tw[:], 0.0)
            twd = pexp.tile([1, 8], BF16, tag="sg2", bufs=4, name="twd")
            nc.scalar.activation(twd[:], tw[:], AF.Exp)

            # ---- HAM warmup: memset-based (no DMA dep), then on wk_s ----
            wsrc = pexp.tile([P, 512], BF16, tag="wsrc", bufs=1, name="wsrc")
            nc.vector.memset(wsrc[:], 0.0)
            warm = ypp.tile([P, 512], F32, tag="y", name="warm")
            for i in range(8):
                nc.tensor.matmul(
                    warm[:], lhsT=wsrc[:, 0:P], rhs=wsrc[:],
                    start=True, stop=True,
                )
            for i in range(8):
                nc.tensor.matmul(
                    warm[:], lhsT=wk_s[:, 0:P], rhs=wk_s[:, 0:512],
                    start=True, stop=True,
                )
            wdump = pexp.tile([P, 4], F32, tag="rc", bufs=6, name="wdump")
            nc.scalar.copy(wdump[:], warm[:, 0:4])

            # ---- projections ----
            def proj_pair(w_s, dst, nt):
                ps = stp.tile([P, 1024], F32, tag="st", name=f"pp{dst.name}")
                for j in range(KJ):
                    nc.tensor.matmul(
                        ps[:, 0:512],
                        lhsT=w_s[:, j * P : (j + 1) * P],
                        rhs=xT[nt][:, j, :],
                        start=(j == 0),
                        stop=(j == KJ - 1),
                    )
                nc.scalar.copy(dst[:], ps[:, 0:512])

            def proj_c(nt):
                # head C: one full-array matmul per j; out rows 0-63 = Q^T,
                # 64-127 = K^T. SBUF->SBUF DMAs duplicate the halves so the
                # C score matmuls can process two key tiles per slot.
                ps = stp.tile([P, 1024], F32, tag="st", name=f"pqk{nt}")
                for j in range(KJ):
                    nc.tensor.matmul(
                        ps[:, 512:1024],
                        lhsT=wc_s[:, j * P : (j + 1) * P],
                        rhs=xT[nt][:, j, :],
                        start=(j == 0),
                        stop=(j == KJ - 1),
                    )
                nc.vector.tensor_copy(qTc[nt][0:64, :], ps[0:64, 512:1024])
                nc.vector.tensor_copy(kTc[nt][64:128, :], ps[64:128, 512:1024])
                nc.scalar.dma_start(qTc[nt][64:128, :], qTc[nt][0:64, :])
                nc.scalar.dma_start(kTc[nt][0:64, :], kTc[nt][64:128, :])

            def proj_v2(t0):
                # two token tiles t0, t0+1 share one 2-bank psum tile
                ps = stp.tile([P, 1024], F32, tag="st", name=f"psv{t0}")
                for u in range(2):
                    t = t0 + u
                    for j in range(KJ):
                        nc.tensor.matmul(
                            ps[:, u * 512 : u * 512 + VW],
                            lhsT=xT[t // 4][:, j, (t % 4) * P : (t % 4 + 1) * P],
                            rhs=wv_s[:, j * VW : (j + 1) * VW],
                            start=(j == 0),
                            stop=(j == KJ - 1),
                        )
                for u in range(2):
                    t = t0 + u
                    src = ps[:, u * 512 : u * 512 + VW].rearrange(
                        "p (h c) -> p h c", c=DH + 1
                    )
                    dst = vsb[t][:].rearrange("p (h c) -> p h c", c=P)[:, :, 0 : DH + 1]
                    nc.scalar.copy(dst, src)
                    ones_ap = vsb[t][:].rearrange("p (h c) -> p h c", c=P)[:, :, DH : DH + 1]
                    nc.gpsimd.memset(ones_ap, 1.0)

            for nt in range(NB):
                proj_pair(wk_s, kTp[nt], nt)
                proj_c(nt)
                proj_v2(4 * nt)
                proj_v2(4 * nt + 2)
                proj_pair(wq_s, qTp[nt], nt)

            def exp_act(dst, src):
                nc.scalar.activation(dst[:], src[:], AF.Exp)

            def exp_dve(dst, src):
                nc.vector.tensor_scalar(
                    dst[:].bitcast(I16), src[:], EXP_C1, EXP_C2,
                    op0=ALU.mult, op1=ALU.add,
                )

            ysb_tiles = {}

            def y_unit(qb, qt, piece, c_first=False):
                # one quarter-tile, half-width piece of the partial
                # y-projection for block qb: pair+C accumulate in one PSUM
                # chain. C-matmuls of consecutive units alternate row
                # groups (wpc/otC rows are duplicated) so they overlap.
                rg = (2 * qt + piece) % 2
                lp = otP[qb][:, qt * P : (qt + 1) * P]
                lc = otC[qb][rg * 64 : (rg + 1) * 64, qt * P : (qt + 1) * P]
                wpcr = wpc_s[rg * 64 : (rg + 1) * 64, :]
                o0 = piece * 384
                if piece == 0:
                    ysb_tiles[(qb, qt)] = pexp.tile(
                        [P, C], BF16, tag="y", bufs=4, name=f"ysb{qb}_{qt}"
                    )
                ysb = ysb_tiles[(qb, qt)]
                ps = ypp.tile([P, 512], F32, tag="y", name=f"psy{qb}_{qt}_{piece}")
                if c_first:
                    nc.tensor.matmul(
                        ps[:, 0:384], lhsT=lc, rhs=wpcr[:, o0 : o0 + 384],
                        start=True, stop=False, tile_position=(rg * 64, 0),
                    )
                    nc.tensor.matmul(
                        ps[:, 0:384], lhsT=lp, rhs=wpp_s[:, o0 : o0 + 384],
                        start=False, stop=True,
                    )
                else:
                    nc.tensor.matmul(
                        ps[:, 0:384], lhsT=lp, rhs=wpp_s[:, o0 : o0 + 384],
                        start=True, stop=False,
                    )
                    nc.tensor.matmul(
                        ps[:, 0:384], lhsT=lc, rhs=wpcr[:, o0 : o0 + 384],
                        start=False, stop=True, tile_position=(rg * 64, 0),
                    )
                if piece == 0:
                    nc.scalar.copy(ysb[:, o0 : o0 + 384], ps[:, 0:384])
                else:
                    nc.vector.tensor_copy(ysb[:, o0 : o0 + 384], ps[:, 0:384])
                    nc.sync.dma_start(
                        out[qb * 512 + qt * P : qb * 512 + (qt + 1) * P, :], ysb[:]
                    )
                    del ysb_tiles[(qb, qt)]

            def weave_y(ys):
                if ys:
                    yu = ys.pop()
                    if yu is not None:
                        y_unit(*yu)

            def normalize_mul(ot, rcrow, dst_ap, tag):
                rb = pexp.tile([64, 512], F32, tag="rb", bufs=6, name=f"rb{tag}")
                nc.gpsimd.partition_broadcast(rb[:], rcrow)
                nc.vector.tensor_mul(dst_ap, ot[0:64, :], rb[:])

            def attention(qb, ys, c_first=False):
                if c_first:
                    attention_c(qb, ys)
                # ---- phase AB ----
                otA = otp.tile([P, 512], F32, tag="ot", name=f"otA{qb}")
                otB = otp.tile([P, 512], F32, tag="ot", name=f"otB{qb}")
                prev = None
                for g in range(KT // 2 + 1):
                    if g < KT // 2:
                        sts = []
                        pps = []
                        for u in range(2):
                            kt = 2 * g + u
                            st = stp.tile([P, 1024], F32, tag="st", name=f"s{qb}_{kt}")
                            nc.tensor.matmul(
                                st[:, 0:512],
                                lhsT=kslice(kTp, kt)[0:64, :],
                                rhs=qTp[qb][0:64, :],
                                start=True, stop=True,
                                tile_position=(0, 0),
                            )
                            nc.tensor.matmul(
                                st[:, 512:1024],
                                lhsT=kslice(kTp, kt)[64:128, :],
                                rhs=qTp[qb][64:128, :],
                                start=True, stop=True,
                                tile_position=(64, 0),
                            )
                            sts.append(st)
                            pp = pexp.tile([P, 1024], BF16, tag="pexp", name=f"p{qb}_{kt}")
                            (exp_act if u == 0 else exp_dve)(pp, st)
                            pps.append(pp)
                    if prev is not None:
                        pg, ppps = prev
                        weave_y(ys)
                        for u in range(2):
                            kt = 2 * pg + u
                            nc.tensor.matmul(
                                otA[:],
                                lhsT=vhead(kt, 0),
                                rhs=ppps[u][:, 0:512],
                                start=(kt == 0), stop=(kt == KT - 1),
                            )
                            nc.tensor.matmul(
                                otB[:],
                                lhsT=vhead(kt, 1),
                                rhs=ppps[u][:, 512:1024],
                                start=(kt == 0), stop=(kt == KT - 1),
                            )
                        weave_y(ys)
                    prev = (g, pps) if g < KT // 2 else None
                # denominators: stage rows side-by-side in the free dim,
                # one batched reciprocal
                sgAB = pexp.tile([1, 1024], F32, tag="sg", bufs=4, name=f"sgAB{qb}")
                nc.scalar.copy(sgAB[0:1, 0:512], otA[64:65, :])
                nc.vector.tensor_copy(sgAB[0:1, 512:1024], otB[64:65, :])
                rcAB = pexp.tile([1, 1024], F32, tag="rc2", bufs=4, name=f"rcAB{qb}")
                nc.vector.reciprocal_approx_fast(rcAB[:], sgAB[:])
                normalize_mul(otA, rcAB[0:1, 0:512], otP[qb][0:64, :], f"A{qb}")
                normalize_mul(otB, rcAB[0:1, 512:1024], otP[qb][64:128, :], f"B{qb}")
                if not c_first:
                    attention_c(qb, ys)

            def attention_c(qb, ys):
                otCc = otp.tile([P, 512], F32, tag="ot", name=f"otC{qb}")
                prev = None
                for g in range(KT // 2 + 1):
                    if g < KT // 2:
                        stC = stp.tile([P, 1024], F32, tag="st", name=f"sC{qb}_{g}")
                        nc.tensor.matmul(
                            stC[:, 0:512],
                            lhsT=kslice(kTc, 2 * g)[0:64, :],
                            rhs=qTc[qb][0:64, :],
                            start=True, stop=True,
                            tile_position=(0, 0),
                        )
                        nc.tensor.matmul(
                            stC[:, 512:1024],
                            lhsT=kslice(kTc, 2 * g + 1)[64:128, :],
                            rhs=qTc[qb][64:128, :],
                            start=True, stop=True,
                            tile_position=(64, 0),
                        )
                        pc = pexp.tile([P, 1024], BF16, tag="pexp", name=f"pc{qb}_{g}")
                        # ACT takes 6 of 8 C tiles (DVE carries normalize)
                        (exp_act if g % 4 != 3 else exp_dve)(pc, stC)
                    if prev is not None:
                        pg, ppc = prev
                        weave_y(ys)
                        for u in range(2):
                            kt = 2 * pg + u
                            nc.tensor.matmul(
                                otCc[:],
                                lhsT=vhead(kt, 2),
                                rhs=ppc[:, u * 512 : (u + 1) * 512],
                                start=(kt == 0), stop=(kt == KT - 1),
                            )
                        weave_y(ys)
                    prev = (g, pc) if g < KT // 2 else None
                sgC = pexp.tile([1, 512], F32, tag="sg2", bufs=4, name=f"sgC{qb}")
                nc.scalar.copy(sgC[:], otCc[64:65, :])
                rcC = pexp.tile([1, 512], F32, tag="rc", bufs=6, name=f"rcC{qb}")
                nc.vector.reciprocal_approx_fast(rcC[:], sgC[:])
                normalize_mul(otCc, rcC[0:1, :], otC[qb][0:64, :], f"C{qb}")
                nc.scalar.dma_start(otC[qb][64:128, :], otC[qb][0:64, :])

            ymap = {2: [0], 3: [1, 2]}
            for qb in range(NB):
                units = [
                    (src_qb, qt, pc)
                    for src_qb in ymap.get(qb, [])
                    for qt in range(4)
                    for pc in range(2)
                ]
                attention(qb, list(reversed(units)), c_first=(qb == NB - 1))

            # tail: block 3's y units (C-part first inside each chain so the
            # C matmuls overlap the A/B normalize)
            lastq = NB - 1
            for qt in range(4):
                for pc in range(2):
                    y_unit(lastq, qt, pc, c_first=True)

    nc.compile()
    return nc


def _get_nc():
    global _nc_cache
    if _nc_cache is None:
        _nc_cache = _build_nc()
    return _nc_cache


def _ktile_major(w):
    # [C, M] -> [128, KJ*M] with contraction tile j at free offset j*M
    M = w.shape[1]
    return np.ascontiguousarray(
        w.reshape(KJ, P, M).transpose(1, 0, 2).reshape(P, KJ * M)
    )


def kernel(x, w_qkv, gate, w_proj, b_proj):
    from concourse import bass_utils

    global LAST_RESULT

    x = np.asarray(x, dtype=np.float32)
    w_qkv = np.asarray(w_qkv, dtype=np.float32)
    gate = np.asarray(gate, dtype=np.float32)
    w_proj = np.asarray(w_proj, dtype=np.float32)
    b_proj = np.asarray(b_proj, dtype=np.float32)

    # ---- host-side layout prep (weights folded/sliced, layout-only for x) ----
    wq_full = w_qkv[:, 0:C] * SCALE
    wk_full = w_qkv[:, C : 2 * C]
    wv_full = w_qkv[:, 2 * C : 3 * C]
    gated_wp = w_proj * np.repeat(gate, DH)[:, None]

    per_hh = []
    for hh in range(4):
        h0 = HL * hh
        ab = slice(h0 * DH, (h0 + 2) * DH)
        cc = slice((h0 + 2) * DH, (h0 + 3) * DH)
        wq_np = _ktile_major(wq_full[:, ab]).astype(_BF16)
        wk_np = _ktile_major(wk_full[:, ab]).astype(_BF16)
        wc_np = _ktile_major(
            np.concatenate([wq_full[:, cc], wk_full[:, cc]], axis=1)
        ).astype(_BF16)
        wv_pad = np.zeros((C, VW), dtype=np.float32)
        for h in range(HL):
            wv_pad[:, h * (DH + 1) : h * (DH + 1) + DH] = wv_full[
                :, (h0 + h) * DH : (h0 + h + 1) * DH
            ]
        wv_np = _ktile_major(wv_pad).astype(_BF16)
        cs = slice(h0 * DH, (h0 + HL) * DH)
        wp_rows = gated_wp[cs, :]  # [192, 768]
        wpp_np = np.ascontiguousarray(wp_rows[0 : 2 * DH, :]).astype(_BF16)
        wpc_half = wp_rows[2 * DH :, :]
        wpc_np = np.ascontiguousarray(np.vstack([wpc_half, wpc_half])).astype(_BF16)
        per_hh.append((wq_np, wk_np, wc_np, wv_np, wpp_np, wpc_np))

    xt_b = []
    for b in range(B):
        xtb = x[b].T.astype(_BF16)  # [C, N]
        xt_b.append(
            [
                _ktile_major(np.ascontiguousarray(xtb[:, n * 512 : (n + 1) * 512]))
                for n in range(NB)
            ]
        )

    in_maps = []
    for c in range(NCORES):
        b, hh = c // 4, c % 4
        wq_np, wk_np, wc_np, wv_np, wpp_np, wpc_np = per_hh[hh]
        m = {f"xt{n}": xt_b[b][n] for n in range(NB)}
        m.update(
            {"wq": wq_np, "wk": wk_np, "wc": wc_np, "wv": wv_np,
             "wpp": wpp_np, "wpc": wpc_np}
        )
        in_maps.append(m)

    nc = _get_nc()
    # the first execution of a freshly compiled NEFF occasionally hits a
    # transient NRT_EXEC_UNIT_UNRECOVERABLE; a retry reliably succeeds
    last_exc = None
    for _attempt in range(3):
        try:
            res = bass_utils.run_bass_kernel_spmd(
                nc, in_maps, core_ids=list(range(NCORES)), trace=TRACE
            )
            break
        except Exception as e:  # noqa: BLE001
            last_exc = e
    else:
        raise last_exc
    LAST_RESULT = res

    out = np.empty((B, N, C), dtype=np.float32)
    for b in range(B):
        acc = np.asarray(res.results[4 * b + 0]["out"]).astype(np.float32)
        for hh in range(1, 4):
            acc += np.asarray(res.results[4 * b + hh]["out"]).astype(np.float32)
        out[b] = acc + b_proj[None, :]
    return out


# revision 16
# speedup vs baseline: 1.0062x; 1.0062x over previous
"""Bass/Trainium2 kernel for nn_Attention (B=2, N=2048, C=768, H=12).

Sharding: 8 cores = 2 batches x 4 head-triples. Core (b, hh) computes
Q/K/V projections for heads {3hh, 3hh+1, 3hh+2} over the full 2048-token
sequence of batch b, attention for those heads, and the partial output
projection y_partial = (attn_out * gate) @ w_proj[rows of those heads].
Host sums the 4 fp32 partials per batch and adds b_proj.

v3 schedule (driven by ntff traces of v1/v2):
- Score tiles are [128, 1024] (2 PSUM banks): A-scores of key tile kt in
  the low half, B-scores of the same kt in the high half, so the two
  score matmuls of a tile are the natural (0,0)/(64,0) row-group pair
  (concurrent fills on separate XBUSes) and each exp instruction covers
  1024 els/partition (halves the fixed cost of ACT/DVE).
- V tiles are padded to [128, 128] per head (ones col at 64, junk
  above): 128-column stationary tiles get Fast Weight Load.
- y projection accumulates pair+C pieces in one PSUM chain and DMAs the
  [128, 384] fp32 result straight to DRAM (out is fp32; host sums) --
  no ysb staging copies, no tail add/copy ladder.
- normalize: denominator rows (PSUM row 64) are DMA'd to SBUF (DMA
  queue, not ACT/DVE), reciprocals batched (A+B in one DVE instr),
  broadcast on GpSimd, one [64,512] mul per head evacuates/normalizes.
- exp split: per block 24 [128,1024] tiles; ACT takes 14, DVE 10
  (DVE also carries normalize) -- both land at the PE floor.
- Warmup matmuls on a memset tile (no DMA dependency) plus a few on
  wk_s once it lands keep HAM at 8/8 through the projections; the ACT
  Exp table preloads during the DMA window.
"""

import numpy as np
import ml_dtypes

B, N, C = 2, 2048, 768
H = 12
DH = C // H
SCALE = DH**-0.5
P = 128
HL = 3  # heads per core
KJ = C // P  # 6 contraction tiles over C
KT = N // P  # 16 key tiles
NB = N // 512  # 4 query blocks / x chunks
VW = HL * (DH + 1)  # 195 v columns (ones col per head)

EXP_C1 = 128.0 / float(np.log(2.0))
EXP_C2 = 16256.0 - 5.5

NCORES = 8
TRACE = False  # test.py flips this to profile
LAST_RESULT = None

_BF16 = ml_dtypes.bfloat16

_nc_cache = None


def _build_nc():
    from contextlib import ExitStack

    import concourse.tile as tile
    from concourse import bacc, mybir

    dt = mybir.dt
    F32, BF16, I16 = dt.float32, dt.bfloat16, dt.int16
    AF = mybir.ActivationFunctionType
    ALU = mybir.AluOpType

    nc = bacc.Bacc("TRN2", target_bir_lowering=False, num_devices=NCORES)

    xt = [
        nc.dram_tensor(f"xt{n}", [P, KJ * 512], BF16, kind="ExternalInput")
        for n in range(NB)
    ]
    wq = nc.dram_tensor("wq", [P, KJ * P], BF16, kind="ExternalInput")
    wk = nc.dram_tensor("wk", [P, KJ * P], BF16, kind="ExternalInput")
    wc = nc.dram_tensor("wc", [P, KJ * P], BF16, kind="ExternalInput")
    wv = nc.dram_tensor("wv", [P, KJ * VW], BF16, kind="ExternalInput")
    wpp = nc.dram_tensor("wpp", [P, C], BF16, kind="ExternalInput")  # pair rows
    wpc = nc.dram_tensor("wpc", [P, C], BF16, kind="ExternalInput")  # head C rows x2
    out = nc.dram_tensor("out", [N, C], BF16, kind="ExternalOutput")

    with tile.TileContext(nc) as tc, ExitStack() as ctx:
        ps_pool = ctx.enter_context(tc.tile_pool(name="persist", bufs=1))

        xT = [
            ps_pool.tile([P, KJ, 512], BF16, tag=f"xT{n}", name=f"xT{n}")
            for n in range(NB)
        ]
        wq_s = ps_pool.tile([P, KJ * P], BF16, tag="wq")
        wk_s = ps_pool.tile([P, KJ * P], BF16, tag="wk")
        wc_s = ps_pool.tile([P, KJ * P], BF16, tag="wc")
        wv_s = ps_pool.tile([P, KJ * VW], BF16, tag="wv")
        wpp_s = ps_pool.tile([P, C], BF16, tag="wpp")
        wpc_s = ps_pool.tile([P, C], BF16, tag="wpc")
        qTp = [ps_pool.tile([P, 512], BF16, tag=f"qTp{n}", name=f"qTp{n}") for n in range(NB)]
        kTp = [ps_pool.tile([P, 512], BF16, tag=f"kTp{n}", name=f"kTp{n}") for n in range(NB)]
        qTc = [ps_pool.tile([P, 512], BF16, tag=f"qTc{n}", name=f"qTc{n}") for n in range(NB)]
        kTc = [ps_pool.tile([P, 512], BF16, tag=f"kTc{n}", name=f"kTc{n}") for n in range(NB)]
        # V: per key tile, [128, 3*128]: head h at cols h*128..h*128+64
        # (64 dh + ones col); cols 65-127 of each head are never-read junk
        vsb = [ps_pool.tile([P, HL * P], BF16, tag=f"v{t}", name=f"v{t}") for t in range(KT)]
        otP = [
            ps_pool.tile([P, 512], BF16, tag=f"otP{q}", name=f"otP{q}") for q in range(NB)
        ]
        otC = [
            ps_pool.tile([P, 512], BF16, tag=f"otC{q}", name=f"otC{q}") for q in range(NB)
        ]

        def kslice(kTx, kt):
            return kTx[kt // 4][:, (kt % 4) * P : (kt % 4 + 1) * P]

        def vhead(t, h):
            return vsb[t][:, h * P : (h + 1) * P]

        # ---- input loads (one HWDGE ring, FIFO) ----
        nc.sync.dma_start(wk_s[:], wk[:])
        nc.sync.dma_start(xT[0][:], xt[0][:].rearrange("p (j n) -> p j n", n=512))
        nc.sync.dma_start(wc_s[:], wc[:])
        nc.sync.dma_start(wv_s[:], wv[:])
        nc.sync.dma_start(wq_s[:], wq[:])
        nc.sync.dma_start(xT[1][:], xt[1][:].rearrange("p (j n) -> p j n", n=512))
        nc.sync.dma_start(xT[2][:], xt[2][:].rearrange("p (j n) -> p j n", n=512))
        nc.sync.dma_start(xT[3][:], xt[3][:].rearrange("p (j n) -> p j n", n=512))
        nc.sync.dma_start(wpp_s[:], wpp[:])
        nc.sync.dma_start(wpc_s[:], wpc[:])

        with (
            tc.tile_pool(name="st", bufs=2, space="PSUM") as stp,
            tc.tile_pool(name="ot", bufs=3, space="PSUM") as otp,
            tc.tile_pool(name="yp", bufs=1, space="PSUM") as ypp,
            tc.tile_pool(name="pexp", bufs=12) as pexp,
        ):
            # ---- ACT Exp table preload (hidden under the DMA window) ----
            tw = pexp.tile([1, 8], F32, tag="rc", bufs=6, name="twarm")
            nc.vector.memset(tw[:], 0.0)
            twd = pexp.tile([1, 8], BF16, tag="sg2", bufs=4, name="twd")
            nc.scalar.activation(twd[:], tw[:], AF.Exp)

            # ---- HAM warmup: memset-based (no DMA dep), then on wk_s ----
            wsrc = pexp.tile([P, 512], BF16, tag="wsrc", bufs=1, name="wsrc")
            nc.vector.memset(wsrc[:], 0.0)
            warm = ypp.tile([P, 512], F32, tag="y", name="warm")
            for i in range(8):
                nc.tensor.matmul(
                    warm[:], lhsT=wsrc[:, 0:P], rhs=wsrc[:],
                    start=True, stop=True,
                )
            for i in range(8):
                nc.tensor.matmul(
                    warm[:], lhsT=wk_s[:, 0:P], rhs=wk_s[:, 0:512],
                    start=True, stop=True,
                )
            wdump = pexp.tile([P, 4], F32, tag="rc", bufs=6, name="wdump")
            nc.scalar.copy(wdump[:], warm[:, 0:4])

            # ---- projections ----
            def proj_pair(w_s, dst, nt):
                ps = stp.tile([P, 1024], F32, tag="st", name=f"pp{dst.name}")
                for j in range(KJ):
                    nc.tensor.matmul(
                        ps[:, 0:512],
                        lhsT=w_s[:, j * P : (j + 1) * P],
                        rhs=xT[nt][:, j, :],
                        start=(j == 0),
                        stop=(j == KJ - 1),
                    )
                nc.scalar.copy(dst[:], ps[:, 0:512])

            def proj_c(nt):
                # head C: one full-array matmul per j; out rows 0-63 = Q^T,
                # 64-127 = K^T. SBUF->SBUF DMAs duplicate the halves so the
                # C score matmuls can process two key tiles per slot.
                ps = stp.tile([P, 1024], F32, tag="st", name=f"pqk{nt}")
                for j in range(KJ):
                    nc.tensor.matmul(
                        ps[:, 512:1024],
                        lhsT=wc_s[:, j * P : (j + 1) * P],
                        rhs=xT[nt][:, j, :],
                        start=(j == 0),
                        stop=(j == KJ - 1),
                    )
                nc.vector.tensor_copy(qTc[nt][0:64, :], ps[0:64, 512:1024])
                nc.vector.tensor_copy(kTc[nt][64:128, :], ps[64:128, 512:1024])
                nc.scalar.dma_start(qTc[nt][64:128, :], qTc[nt][0:64, :])
                nc.scalar.dma_start(kTc[nt][0:64, :], kTc[nt][64:128, :])

            def proj_v2(t0):
                # two token tiles t0, t0+1 share one 2-bank psum tile
                ps = stp.tile([P, 1024], F32, tag="st", name=f"psv{t0}")
                for u in range(2):
                    t = t0 + u
                    for j in range(KJ):
                        nc.tensor.matmul(
                            ps[:, u * 512 : u * 512 + VW],
                            lhsT=xT[t // 4][:, j, (t % 4) * P : (t % 4 + 1) * P],
                            rhs=wv_s[:, j * VW : (j + 1) * VW],
                            start=(j == 0),
                            stop=(j == KJ - 1),
                        )
                for u in range(2):
                    t = t0 + u
                    src = ps[:, u * 512 : u * 512 + VW].rearrange(
                        "p (h c) -> p h c", c=DH + 1
                    )
                    dst = vsb[t][:].rearrange("p (h c) -> p h c", c=P)[:, :, 0 : DH + 1]
                    nc.scalar.copy(dst, src)
                    ones_ap = vsb[t][:].rearrange("p (h c) -> p h c", c=P)[:, :, DH : DH + 1]
                    nc.gpsimd.memset(ones_ap, 1.0)

            for nt in range(NB):
                proj_pair(wk_s, kTp[nt], nt)
                proj_c(nt)
                proj_v2(4 * nt)
                proj_v2(4 * nt + 2)
                proj_pair(wq_s, qTp[nt], nt)

            def exp_act(dst, src):
                nc.scalar.activation(dst[:], src[:], AF.Exp)

            def exp_dve(dst, src):
                nc.vector.tensor_scalar(
                    dst[:].bitcast(I16), src[:], EXP_C1, EXP_C2,
                    op0=ALU.mult, op1=ALU.add,
                )

            ysb_tiles = {}

            def y_unit(qb, qt, piece, c_first=False, pool=None):
                # one quarter-tile, half-width piece of the partial
                # y-projection for block qb: pair+C accumulate in one PSUM
                # chain. C-matmuls of consecutive units alternate row
                # groups (wpc/otC rows are duplicated) so they overlap.
                rg = (2 * qt + piece) % 2
                lp = otP[qb][:, qt * P : (qt + 1) * P]
                lc = otC[qb][rg * 64 : (rg + 1) * 64, qt * P : (qt + 1) * P]
                wpcr = wpc_s[rg * 64 : (rg + 1) * 64, :]
                o0 = piece * 384
                if piece == 0:
                    ysb_tiles[(qb, qt)] = pexp.tile(
                        [P, C], BF16, tag="y", bufs=4, name=f"ysb{qb}_{qt}"
                    )
                ysb = ysb_tiles[(qb, qt)]
                if pool is None:
                    ps = ypp.tile([P, 512], F32, tag="y", name=f"psy{qb}_{qt}_{piece}")
                else:
                    ps = pool.tile([P, 1024], F32, tag="st", name=f"psy{qb}_{qt}_{piece}")
                if c_first:
                    nc.tensor.matmul(
                        ps[:, 0:384], lhsT=lc, rhs=wpcr[:, o0 : o0 + 384],
                        start=True, stop=False, tile_position=(rg * 64, 0),
                    )
                    nc.tensor.matmul(
                        ps[:, 0:384], lhsT=lp, rhs=wpp_s[:, o0 : o0 + 384],
                        start=False, stop=True,
                    )
                else:
                    nc.tensor.matmul(
                        ps[:, 0:384], lhsT=lp, rhs=wpp_s[:, o0 : o0 + 384],
                        start=True, stop=False,
                    )
                    nc.tensor.matmul(
                        ps[:, 0:384], lhsT=lc, rhs=wpcr[:, o0 : o0 + 384],
                        start=False, stop=True, tile_position=(rg * 64, 0),
                    )
                if piece == 0:
                    nc.scalar.copy(ysb[:, o0 : o0 + 384], ps[:, 0:384])
                else:
                    nc.vector.tensor_copy(ysb[:, o0 : o0 + 384], ps[:, 0:384])
                    nc.sync.dma_start(
                        out[qb * 512 + qt * P : qb * 512 + (qt + 1) * P, :], ysb[:]
                    )
                    del ysb_tiles[(qb, qt)]

            def weave_y(ys):
                if ys:
                    yu = ys.pop()
                    if yu is not None:
                        y_unit(*yu)

            def normalize_mul(ot, rcrow, dst_ap, tag):
                rb = pexp.tile([64, 512], F32, tag="rb", bufs=6, name=f"rb{tag}")
                nc.gpsimd.partition_broadcast(rb[:], rcrow)
                nc.vector.tensor_mul(dst_ap, ot[0:64, :], rb[:])

            def attention(qb, ys, c_first=False):
                if c_first:
                    attention_c(qb, ys)
                # ---- phase AB ----
                otA = otp.tile([P, 512], F32, tag="ot", name=f"otA{qb}")
                otB = otp.tile([P, 512], F32, tag="ot", name=f"otB{qb}")
                prev = None
                for g in range(KT // 2 + 1):
                    if g < KT // 2:
                        sts = []
                        pps = []
                        for u in range(2):
                            kt = 2 * g + u
                            st = stp.tile([P, 1024], F32, tag="st", name=f"s{qb}_{kt}")
                            nc.tensor.matmul(
                                st[:, 0:512],
                                lhsT=kslice(kTp, kt)[0:64, :],
                                rhs=qTp[qb][0:64, :],
                                start=True, stop=True,
                                tile_position=(0, 0),
                            )
                            nc.tensor.matmul(
                                st[:, 512:1024],
                                lhsT=kslice(kTp, kt)[64:128, :],
                                rhs=qTp[qb][64:128, :],
                                start=True, stop=True,
                                tile_position=(64, 0),
                            )
                            sts.append(st)
                            pp = pexp.tile([P, 1024], BF16, tag="pexp", name=f"p{qb}_{kt}")
                            (exp_act if u == 0 else exp_dve)(pp, st)
                            pps.append(pp)
                    if prev is not None:
                        pg, ppps = prev
                        weave_y(ys)
                        for u in range(2):
                            kt = 2 * pg + u
                            nc.tensor.matmul(
                                otA[:],
                                lhsT=vhead(kt, 0),
                                rhs=ppps[u][:, 0:512],
                                start=(kt == 0), stop=(kt == KT - 1),
                            )
                            nc.tensor.matmul(
                                otB[:],
                                lhsT=vhead(kt, 1),
                                rhs=ppps[u][:, 512:1024],
                                start=(kt == 0), stop=(kt == KT - 1),
                            )
                        weave_y(ys)
                    prev = (g, pps) if g < KT // 2 else None
                # denominators: stage rows side-by-side in the free dim,
                # one batched reciprocal
                sgAB = pexp.tile([1, 1024], F32, tag="sg", bufs=4, name=f"sgAB{qb}")
                nc.scalar.copy(sgAB[0:1, 0:512], otA[64:65, :])
                nc.vector.tensor_copy(sgAB[0:1, 512:1024], otB[64:65, :])
                rcAB = pexp.tile([1, 1024], F32, tag="rc2", bufs=4, name=f"rcAB{qb}")
                nc.vector.reciprocal_approx_fast(rcAB[:], sgAB[:])
                normalize_mul(otA, rcAB[0:1, 0:512], otP[qb][0:64, :], f"A{qb}")
                normalize_mul(otB, rcAB[0:1, 512:1024], otP[qb][64:128, :], f"B{qb}")
                if not c_first:
                    attention_c(qb, ys)

            def attention_c(qb, ys):
                otCc = otp.tile([P, 512], F32, tag="ot", name=f"otC{qb}")
                prev = None
                for g in range(KT // 2 + 1):
                    if g < KT // 2:
                        stC = stp.tile([P, 1024], F32, tag="st", name=f"sC{qb}_{g}")
                        nc.tensor.matmul(
                            stC[:, 0:512],
                            lhsT=kslice(kTc, 2 * g)[0:64, :],
                            rhs=qTc[qb][0:64, :],
                            start=True, stop=True,
                            tile_position=(0, 0),
                        )
                        nc.tensor.matmul(
                            stC[:, 512:1024],
                            lhsT=kslice(kTc, 2 * g + 1)[64:128, :],
                            rhs=qTc[qb][64:128, :],
                            start=True, stop=True,
                            tile_position=(64, 0),
                        )
                        pc = pexp.tile([P, 1024], BF16, tag="pexp", name=f"pc{qb}_{g}")
                        # ACT takes 6 of 8 C tiles (DVE carries normalize)
                        (exp_act if g % 4 != 3 else exp_dve)(pc, stC)
                    if prev is not None:
                        pg, ppc = prev
                        weave_y(ys)
                        for u in range(2):
                            kt = 2 * pg + u
                            nc.tensor.matmul(
                                otCc[:],
                                lhsT=vhead(kt, 2),
                                rhs=ppc[:, u * 512 : (u + 1) * 512],
                                start=(kt == 0), stop=(kt == KT - 1),
                            )
                        weave_y(ys)
                    prev = (g, pc) if g < KT // 2 else None
                sgC = pexp.tile([1, 512], F32, tag="sg2", bufs=4, name=f"sgC{qb}")
                nc.scalar.copy(sgC[:], otCc[64:65, :])
                rcC = pexp.tile([1, 512], F32, tag="rc", bufs=6, name=f"rcC{qb}")
                nc.vector.reciprocal_approx_fast(rcC[:], sgC[:])
                normalize_mul(otCc, rcC[0:1, :], otC[qb][0:64, :], f"C{qb}")
                nc.scalar.dma_start(otC[qb][64:128, :], otC[qb][0:64, :])

            ymap = {2: [0], 3: [1, 2]}
            for qb in range(NB):
                units = [
                    (src_qb, qt, pc)
                    for src_qb in ymap.get(qb, [])
                    for qt in range(4)
                    for pc in range(2)
                ]
                attention(qb, list(reversed(units)), c_first=(qb == NB - 1))

            # tail: block 3's y units (C-part first inside each chain so the
            # C matmuls overlap the A/B normalize); psum from the st pool
            # (free at the tail) so 3 chains are in flight
            lastq = NB - 1
            for qt in range(4):
                for pc in range(2):
                    y_unit(lastq, qt, pc, c_first=True, pool=stp)

    nc.compile()
    return nc


def _get_nc():
    global _nc_cache
    if _nc_cache is None:
        _nc_cache = _build_nc()
    return _nc_cache


def _ktile_major(w):
    # [C, M] -> [128, KJ*M] with contraction tile j at free offset j*M
    M = w.shape[1]
    return np.ascontiguousarray(
        w.reshape(KJ, P, M).transpose(1, 0, 2).reshape(P, KJ * M)
    )


_ldw_patched = False


def _enable_ldw_opt():
    # walrus's LDWEIGHTS optimizer is hardcoded off in bass_utils; the
    # per-matmul weight load is this kernel's main PE overhead, so flip
    # the flag for our compile (results are verified against the
    # reference regardless).
    global _ldw_patched
    if _ldw_patched:
        return
    from concourse import bass_utils

    orig = bass_utils.run_command

    def patched(cmd, *a, **kw):
        if isinstance(cmd, list):
            cmd = [
                "--enable-ldw-opt=true" if c == "--enable-ldw-opt=false" else c
                for c in cmd
            ]
        return orig(cmd, *a, **kw)

    bass_utils.run_command = patched
    _ldw_patched = True


def kernel(x, w_qkv, gate, w_proj, b_proj):
    from concourse import bass_utils

    global LAST_RESULT
    # note: _enable_ldw_opt() breaks walrus codegen (visitInstLdweights
    # error) -- the flag is off upstream for a reason; do not call it.

    x = np.asarray(x, dtype=np.float32)
    w_qkv = np.asarray(w_qkv, dtype=np.float32)
    gate = np.asarray(gate, dtype=np.float32)
    w_proj = np.asarray(w_proj, dtype=np.float32)
    b_proj = np.asarray(b_proj, dtype=np.float32)

    # ---- host-side layout prep (weights folded/sliced, layout-only for x) ----
    wq_full = w_qkv[:, 0:C] * SCALE
    wk_full = w_qkv[:, C : 2 * C]
    wv_full = w_qkv[:, 2 * C : 3 * C]
    gated_wp = w_proj * np.repeat(gate, DH)[:, None]

    per_hh = []
    for hh in range(4):
        h0 = HL * hh
        ab = slice(h0 * DH, (h0 + 2) * DH)
        cc = slice((h0 + 2) * DH, (h0 + 3) * DH)
        wq_np = _ktile_major(wq_full[:, ab]).astype(_BF16)
        wk_np = _ktile_major(wk_full[:, ab]).astype(_BF16)
        wc_np = _ktile_major(
            np.concatenate([wq_full[:, cc], wk_full[:, cc]], axis=1)
        ).astype(_BF16)
        wv_pad = np.zeros((C, VW), dtype=np.float32)
        for h in range(HL):
            wv_pad[:, h * (DH + 1) : h * (DH + 1) + DH] = wv_full[
                :, (h0 + h) * DH : (h0 + h + 1) * DH
            ]
        wv_np = _ktile_major(wv_pad).astype(_BF16)
        cs = slice(h0 * DH, (h0 + HL) * DH)
        wp_rows = gated_wp[cs, :]  # [192, 768]
        wpp_np = np.ascontiguousarray(wp_rows[0 : 2 * DH, :]).astype(_BF16)
        wpc_half = wp_rows[2 * DH :, :]
        wpc_np = np.ascontiguousarray(np.vstack([wpc_half, wpc_half])).astype(_BF16)
        per_hh.append((wq_np, wk_np, wc_np, wv_np, wpp_np, wpc_np))

    xt_b = []
    for b in range(B):
        xtb = x[b].T.astype(_BF16)  # [C, N]
        xt_b.append(
            [
                _ktile_major(np.ascontiguousarray(xtb[:, n * 512 : (n + 1) * 512]))
                for n in range(NB)
            ]
        )

    in_maps = []
    for c in range(NCORES):
        b, hh = c // 4, c % 4
        wq_np, wk_np, wc_np, wv_np, wpp_np, wpc_np = per_hh[hh]
        m = {f"xt{n}": xt_b[b][n] for n in range(NB)}
        m.update(
            {"wq": wq_np, "wk": wk_np, "wc": wc_np, "wv": wv_np,
             "wpp": wpp_np, "wpc": wpc_np}
        )
        in_maps.append(m)

    nc = _get_nc()
    # the first execution of a freshly compiled NEFF occasionally hits a
    # transient NRT_EXEC_UNIT_UNRECOVERABLE; a retry reliably succeeds
    last_exc = None
    for _attempt in range(3):
        try:
            res = bass_utils.run_bass_kernel_spmd(
                nc, in_maps, core_ids=list(range(NCORES)), trace=TRACE
            )
            break
        except Exception as e:  # noqa: BLE001
            last_exc = e
    else:
        raise last_exc
    LAST_RESULT = res

    out = np.empty((B, N, C), dtype=np.float32)
    for b in range(B):
        acc = np.asarray(res.results[4 * b + 0]["out"]).astype(np.float32)
        for hh in range(1, 4):
            acc += np.asarray(res.results[4 * b + hh]["out"]).astype(np.float32)
        out[b] = acc + b_proj[None, :]
    return out
